# revision 1
# baseline (speedup 1.0000x reference)
"""CRF loss (forward-algorithm normalizer + tag-sequence score) on 8 trn2 cores.

Math
----
reference loss = sum_b (orig[y[b,0]] + sum_t trans[y[b,t],y[b,t+1]] - normalizer[b])
normalizer[b]  = sum_j alpha_{tau_b}[j, b],  tau_b = batch_sizes[b]-1
alpha_t[j, b]  = x_t[j, b] + logsumexp_k(alpha_{t-1}[k, b] + trans[j, k]),
alpha_0        = x_0 + orig.

Device recursion runs in the exp domain: with ea_t = exp(alpha_t - D_t[b]),
the step is one matmul + one elementwise multiply:

    S_t  = ETT @ ea_{t-1}              # ETT[k, j] = exp(trans[j, k])
    ea_t = exp(x_t) * S_t * r_t        # r_t = RSCALE * rhat every REN steps
    D_t  = D_{t-1} - ln r_t            # rhat recorded in bf16; recorded ==
                                       # applied exactly, so the approximate
                                       # reciprocal costs nothing in accuracy

Per-core layout stacks two 32-column batch chains on the 128 partitions
(chain a=0 on partitions 0:64, a=1 on 64:128) with a block-diagonal ETT,
so each timestep is a single [128x128]x[128,32] matmul and a single
[128,32] DVE multiply -- the serial chain is latency-bound, so fewer,
uniform instructions per step win.  Renorm prep (sigma matmul from the
te-3 state, fast reciprocal, bf16 record, broadcast matmul, w-scale) is
pipelined over the 3 steps before each event so it never blocks the chain.

The tag-score side is computed from host-built integer histograms
(count[next, cur] of transition pairs, plus first-tag counts): the device
does sum(count * ptab) where ptab = [trans^T | orig]; parameters are only
ever touched on device.

Sharding: data-parallel over batch, 64 rows per core; per-core partial
sums combined on the host (pure index constants only).
"""

import sys

sys.path.insert(0, "/opt/trn_rl_repo")

import numpy as np
import ml_dtypes

import concourse.bass as bass
import concourse.tile as tile
from concourse import bacc, mybir
from concourse.bass_utils import run_bass_kernel_spmd

# Problem constants (hardcoded per the task contract).
B, T, C = 512, 512, 64
M = 8            # cores
BL = B // M      # 64 batch rows per core
GW = 32          # columns per chain (2 chains stacked on partitions)
REN = 8          # renormalize every REN steps
NEVT = T // REN - 1          # 63 renorm events (t = 8, 16, ..., 504)
RQ = NEVT        # one [2, GW] recip-history block per event, partition base 0
RSCALE = 2.0 ** -40          # renorm down-scale, exact in bf16
LN_RSCALE = float(-40.0 * np.log(2.0))
CHUNK = 32       # timesteps of x per DMA chunk
SUB = 8          # timesteps per exp sub-activation
PAD_VAL = -1

f32 = mybir.dt.float32
bf16 = mybir.dt.bfloat16
u16 = mybir.dt.uint16
i16 = mybir.dt.int16
AF = mybir.ActivationFunctionType
ALU = mybir.AluOpType

# packed f32 const columns: trT | orig | ptab | cnt | parm
PK_TR, PK_OR, PK_PT, PK_CN, PK_PA = 0, 64, 65, 130, 195
PKF_W = 195 + 2 * BL
# packed bf16 const columns: b2 | ones2
PKB_W = 130

_CACHE = {}


def build_program(bench_reps=1):
    key = ("nc", bench_reps)
    if key in _CACHE:
        return _CACHE[key]
    nc = bacc.Bacc("TRN2", target_bir_lowering=False, debug=False)

    xT = nc.declare_dram_parameter("xT", [128, T * GW], f32, isOutput=False)
    pkf = nc.declare_dram_parameter("pkf", [128, PKF_W], f32, isOutput=False)
    pkb = nc.declare_dram_parameter("pkb", [128, PKB_W], bf16, isOutput=False)
    sidx = nc.declare_dram_parameter("sidx", [128, 4], i16, isOutput=False)
    cutm = nc.declare_dram_parameter("cutm", [128, RQ * GW], f32, isOutput=False)
    res = nc.declare_dram_parameter("res", [1, 8], f32, isOutput=True)

    NCH = T // CHUNK

    with tile.TileContext(nc) as tc:
        with (
            tc.tile_pool(name="const", bufs=1) as const,
            tc.tile_pool(name="hist", bufs=1) as histp,
            tc.tile_pool(name="xc", bufs=3) as xcpool,
            tc.tile_pool(name="xe", bufs=3) as xepool,
            tc.tile_pool(name="w", bufs=2) as wpool,
            tc.tile_pool(name="post", bufs=1) as post,
            tc.tile_pool(name="psS", bufs=2, space="PSUM") as psS,
            tc.tile_pool(name="psSig", bufs=1, space="PSUM") as psSig,
            tc.tile_pool(name="psRb", bufs=1, space="PSUM") as psRb,
            tc.tile_pool(name="psFin", bufs=1, space="PSUM") as psFin,
        ):
            # ---- packed constants (two DMAs; sidx/cutm land later) ----
            pkf_s = const.tile([128, PKF_W], f32, tag="pkf")
            nc.sync.dma_start(pkf_s[:], pkf[:])
            pkb_s = const.tile([128, PKB_W], bf16, tag="pkb")
            nc.sync.dma_start(pkb_s[:], pkb[:])

            trT_v = pkf_s[0:C, PK_TR : PK_TR + C]
            orig_v = pkf_s[:, PK_OR : PK_OR + 1]
            ptab_v = pkf_s[0:C, PK_PT : PK_PT + C + 1]
            cnt_v = pkf_s[0:C, PK_CN : PK_CN + C + 1]
            parm_v = pkf_s[:, PK_PA : PK_PA + 2 * BL]
            b2_v = pkb_s[:, 0:128]
            ones2_v = pkb_s[:, 128:130]

            eo = const.tile([128, 1], f32, tag="eo")
            nc.scalar.activation(eo[:], orig_v, AF.Exp)

            # Block-diagonal exp(trans)^T: chain a on partitions a*64..a*64+63.
            ett = const.tile([128, 128], bf16, tag="ett")
            nc.vector.memset(ett[:], 0.0)
            nc.scalar.activation(ett[0:C, 0:C], trT_v, AF.Exp)
            nc.scalar.activation(ett[C:128, C:128], trT_v, AF.Exp)

            ones_col = const.tile([128, 1], f32, tag="ones_col")
            nc.vector.memset(ones_col[:], 1.0)

            # recip history: event r at partitions 0:2, columns [r*GW,(r+1)*GW).
            # Every slot is written exactly once, so no preset needed.
            rhist = const.tile([2, RQ * GW], bf16, tag="rhist")

            itc = const.tile([1, 1], f32, tag="itc")
            nc.vector.memset(itc[:], 0.0)

            # ea history: step t at columns [t*GW, (t+1)*GW).
            hist = histp.tile([128, T * GW], bf16, tag="hist")

            # ---- tag-score from host histograms (independent of recursion) ----
            gmul = post.tile([C, C + 1], f32, tag="gmul")
            gacc = post.tile([C, 1], f32, tag="gacc")
            nc.vector.scalar_tensor_tensor(
                gmul[:], cnt_v, 1.0, ptab_v,
                op0=ALU.mult, op1=ALU.mult, accum_out=gacc[:],
            )
            btot = psFin.tile([1, 1], f32, tag="Rg")
            nc.tensor.matmul(
                btot[:], ones_col[0:C, :], gacc[:], start=True, stop=True
            )

            def emit_recursion():
                chunk_tiles = {}

                def emit_chunk_dma(ci, split=1):
                    xc = xcpool.tile([128, CHUNK * GW], f32, tag="xc")
                    w = CHUNK * GW // split
                    for s in range(split):
                        nc.sync.dma_start(
                            xc[:, s * w : (s + 1) * w],
                            xT[:, ci * CHUNK * GW + s * w : ci * CHUNK * GW + (s + 1) * w],
                        )
                    xe = xepool.tile([128, CHUNK * GW], bf16, tag="xe")
                    chunk_tiles[ci] = (xc, xe)

                def emit_subexp(blk):
                    # blk covers steps [blk*SUB, (blk+1)*SUB)
                    if blk * SUB >= T:
                        return
                    ci, sub = divmod(blk, CHUNK // SUB)
                    xc, xe = chunk_tiles[ci]
                    lo, hi = sub * SUB * GW, (sub + 1) * SUB * GW
                    nc.scalar.activation(xe[:, lo:hi], xc[:, lo:hi], AF.Exp)

                # chunk 0 lands in 4 sub-DMAs so exp/step-0 start early
                emit_chunk_dma(0, split=2)
                emit_chunk_dma(1)
                for blk in range(3):
                    emit_subexp(blk)

                # ---- t = 0: ea_0 = exp(x_0) * exp(orig) ----
                xe0 = chunk_tiles[0][1]
                nc.vector.tensor_scalar_mul(hist[:, 0:GW], xe0[:, 0:GW], eo[:])

                wt = None
                rr = None
                sig = None
                for t in range(1, T):
                    ci, off = divmod(t, CHUNK)
                    if off == 0 and ci + 1 < NCH:
                        emit_chunk_dma(ci + 1)
                    if t % SUB == 0:
                        emit_subexp(t // SUB + 2)

                    xecur = chunk_tiles[ci][1]
                    xoff = off * GW

                    if t % REN == 0 and t <= REN * NEVT:
                        win = wt[:]
                    else:
                        win = xecur[:, xoff : xoff + GW]

                    S = psS.tile([128, GW], f32, tag="S")
                    nc.tensor.matmul(
                        S[:], ett[:], hist[:, (t - 1) * GW : t * GW],
                        start=True, stop=True,
                    )
                    nc.vector.tensor_mul(hist[:, t * GW : (t + 1) * GW], S[:], win)

                    # renorm prep for the event at te = next multiple of REN,
                    # pipelined over the 3 preceding steps (sigma from te-3).
                    ph = t % REN
                    te = t + (REN - ph)
                    if te <= REN * NEVT:
                        r = te // REN - 1
                        rcol = r * GW
                        if ph == REN - 3:
                            sig = psSig.tile([2, GW], f32, tag="sig")
                            nc.tensor.matmul(
                                sig[:], ones2_v, hist[:, t * GW : (t + 1) * GW],
                                start=True, stop=True,
                            )
                            rr = wpool.tile([2, GW], f32, tag="rr")
                        elif ph == REN - 2:
                            nc.vector.reciprocal_approx_fast(rr[:], sig[:])
                            nc.vector.tensor_copy(
                                rhist[0:2, rcol : rcol + GW], rr[:]
                            )
                        elif ph == REN - 1:
                            Rb = psRb.tile([128, GW], f32, tag="Rb")
                            nc.tensor.matmul(
                                Rb[:],
                                b2_v[0:2, :],
                                rhist[0:2, rcol : rcol + GW],
                                start=True, stop=True,
                            )
                            nci, noff = divmod(t + 1, CHUNK)
                            wt = wpool.tile([128, GW], bf16, tag="wt")
                            nc.vector.tensor_mul(
                                wt[:],
                                chunk_tiles[nci][1][:, noff * GW : (noff + 1) * GW],
                                Rb[:],
                            )

            def emit_rep():
                nc.vector.tensor_scalar_add(itc[:], itc[:], 1.0)
                emit_recursion()

            if bench_reps == 1:
                emit_rep()
            else:
                with tc.For_i(0, bench_reps, 1):
                    emit_rep()

            # ---- tail: recip-history reduction + final extraction ----
            cutm_s = const.tile([128, RQ * GW], f32, tag="cutm")
            nc.sync.dma_start(cutm_s[:], cutm[:])
            sidx_r = const.tile([128, 4], i16, tag="sidx_r")
            nc.sync.dma_start(sidx_r[:], sidx[:])
            # the gpsimd gather wants all inputs written by one engine: DVE stage
            sidx_s = const.tile([128, 4], i16, tag="sidx")
            nc.vector.tensor_copy(sidx_s[:], sidx_r[:])

            # 4 sub-passes: quarters become ready as their events complete,
            # so the scheduler hoists the early ones into recursion slack.
            lnr = post.tile([2, RQ * GW], f32, tag="lnr")
            masked = post.tile([2, RQ * GW], f32, tag="masked")
            racc = post.tile([2, 4], f32, tag="racc")
            NQ4 = (RQ * GW) // 4
            for q in range(4):
                lo, hi = q * NQ4, (q + 1) * NQ4
                nc.scalar.activation(lnr[0:2, lo:hi], rhist[0:2, lo:hi], AF.Ln)
                nc.vector.scalar_tensor_tensor(
                    masked[0:2, lo:hi], lnr[0:2, lo:hi], 1.0, cutm_s[0:2, lo:hi],
                    op0=ALU.mult, op1=ALU.mult,
                    accum_out=racc[0:2, q : q + 1],
                )

            # ap_gather of column PAIRS (bf16 needs d*size % 4 == 0):
            # out pair-slot b holds hist cols 2*pi, 2*pi+1 for pi = idx//2;
            # the wanted column's parity is folded into the parm mask.
            snap = post.tile([128, 2 * BL], bf16, tag="snap")
            nc.gpsimd.ap_gather(
                snap[:].rearrange("p (n d) -> p n d", d=2),
                hist[:].rearrange("p (n d) -> p n d", d=2),
                sidx_s[:],
                channels=128, num_elems=T * GW // 2, d=2, num_idxs=BL,
            )
            snapln = post.tile([128, 2 * BL], f32, tag="snapln")
            nc.scalar.activation(snapln[:], snap[:], AF.Ln)
            snapsel = post.tile([128, 2 * BL], f32, tag="snapsel")
            sacc = post.tile([128, 1], f32, tag="sacc")
            nc.vector.scalar_tensor_tensor(
                snapsel[:], snapln[:], 1.0, parm_v,
                op0=ALU.mult, op1=ALU.mult, accum_out=sacc[:],
            )

            nA = psFin.tile([1, 1], f32, tag="RA")
            nc.tensor.matmul(nA[:], ones_col[:], sacc[:], start=True, stop=True)
            nB = psFin.tile([1, 4], f32, tag="RB")
            nc.tensor.matmul(nB[:], ones_col[0:2, :], racc[:], start=True, stop=True)

            out_s = post.tile([1, 8], f32, tag="out")
            nc.vector.tensor_copy(out_s[0:1, 0:1], btot[:])
            nc.vector.tensor_copy(out_s[0:1, 1:2], nA[:])
            nc.vector.tensor_copy(out_s[0:1, 2:6], nB[:])
            nc.vector.tensor_copy(out_s[0:1, 6:7], itc[:])
            nc.vector.tensor_copy(out_s[0:1, 7:8], itc[:])
            nc.sync.dma_start(res[:], out_s[:])

    nc.compile()
    _CACHE[key] = nc
    return nc


def host_inputs(pad_x, transition_scores, origination_scores, pad_y, batch_sizes):
    """Shard + lay out the full inputs into 8 per-core input maps.

    Host work is limited to data movement and integer index preprocessing;
    every floating-point op on learned parameters / activations runs on
    device.  Returns (in_maps, nev_consts)."""
    pad_x = np.ascontiguousarray(np.asarray(pad_x, dtype=np.float32))
    trans = np.ascontiguousarray(np.asarray(transition_scores, dtype=np.float32))
    origv = np.ascontiguousarray(np.asarray(origination_scores, dtype=np.float32))
    pad_y = np.asarray(pad_y)
    batch_sizes = np.asarray(batch_sizes)

    # x: xT[c][a*64 + k, t*32 + cc] = pad_x[c*64 + a*32 + cc, t, k]
    xr = pad_x.reshape(M, 2, GW, T, C).transpose(0, 1, 4, 3, 2)
    xT = np.ascontiguousarray(xr).reshape(M, 128, T * GW)

    y = np.where(pad_y == PAD_VAL, 0, pad_y).astype(np.int64)
    tau = batch_sizes.astype(np.int64) - 1

    pkb = np.zeros((128, PKB_W), np.float32)
    pkb[0, 0:64] = RSCALE          # b2 row 0 -> chain A
    pkb[1, 64:128] = RSCALE        # b2 row 1 -> chain B
    pkb[0:64, 128] = 1.0           # ones2 col 0
    pkb[64:128, 129] = 1.0         # ones2 col 1
    pkb = np.ascontiguousarray(pkb.astype(ml_dtypes.bfloat16))

    # parm[chain_half(b), 2*b + parity(b)] = 1; gathered pairs elsewhere 0
    parm = np.zeros((128, 2 * BL), np.float32)
    for b in range(BL):
        a = b // GW
        parm[a * 64 : (a + 1) * 64, 2 * b + (b % 2)] = 1.0

    in_maps = []
    nevs = []
    for c in range(M):
        yc = y[c * BL : (c + 1) * BL]
        # count[next, cur] histogram + first-tag histogram (integer only)
        pair = (yc[:, 1:] * C + yc[:, :-1]).reshape(-1)
        cntm = np.bincount(pair, minlength=C * C).astype(np.float32).reshape(C, C)
        ho = np.bincount(yc[:, 0], minlength=C).astype(np.float32).reshape(C, 1)

        pkf = np.zeros((128, PKF_W), np.float32)
        pkf[0:C, PK_TR : PK_TR + C] = trans.T
        pkf[:, PK_OR] = np.concatenate([origv, origv])
        pkf[0:C, PK_PT : PK_PT + C] = trans.T
        pkf[0:C, PK_PT + C] = origv
        pkf[0:C, PK_CN : PK_CN + C] = cntm
        pkf[0:C, PK_CN + C] = ho[:, 0]
        pkf[:, PK_PA : PK_PA + 2 * BL] = parm

        tauc = tau[c * BL : (c + 1) * BL]
        # pair index of column tau*GW + (b%GW); parity = b%2 (GW is even)
        idxp = ((tauc * GW + (np.arange(BL) % GW)) // 2).astype(np.int16)
        blk = idxp.reshape(4, 16).T  # wrapped per 16 partitions
        sidx = np.ascontiguousarray(np.tile(blk, (8, 1)))

        # cutm[a, r*GW+cc] = 1 iff event r has REN*(r+1) <= tau of batch col
        # b = a*GW + cc
        cutm = np.zeros((128, RQ * GW), np.float32)
        for r in range(NEVT):
            t_r = REN * (r + 1)
            for a in range(2):
                bvals = tauc[a * GW : (a + 1) * GW]
                cutm[a, r * GW : (r + 1) * GW] = (t_r <= bvals).astype(np.float32)

        nevs.append(float(np.minimum(tauc // REN, NEVT).sum()))

        in_maps.append(
            {
                "xT": np.ascontiguousarray(xT[c]),
                "pkf": np.ascontiguousarray(pkf),
                "pkb": pkb,
                "sidx": sidx,
                "cutm": np.ascontiguousarray(cutm),
            }
        )
    return in_maps, nevs


def combine(results, nevs):
    total = 0.0
    for r, nev in zip(results, nevs):
        v = np.asarray(r["res"], dtype=np.float64).reshape(-1)
        # loss_core = score - sum_b normalizer_b
        #           = v0 - (v1 - C*(sum(v2..v5) + ln(RSCALE)*nev))
        total += v[0] - v[1] + C * (v[2] + v[3] + v[4] + v[5] + LN_RSCALE * nev)
    return np.asarray(total, dtype=np.float32)


def kernel(pad_x, transition_scores, origination_scores, pad_y, batch_sizes):
    nc = build_program()
    in_maps, nevs = host_inputs(
        pad_x, transition_scores, origination_scores, pad_y, batch_sizes
    )
    out = run_bass_kernel_spmd(nc, in_maps, core_ids=list(range(M)))
    return combine(out.results, nevs)



# revision 11
# speedup vs baseline: 1.0105x; 1.0105x over previous
"""CRF loss (forward-algorithm normalizer + tag-sequence score) on 8 trn2 cores.

Math
----
reference loss = sum_b (orig[y[b,0]] + sum_t trans[y[b,t],y[b,t+1]] - normalizer[b])
normalizer[b]  = sum_j alpha_{tau_b}[j, b],  tau_b = batch_sizes[b]-1
alpha_t[j, b]  = x_t[j, b] + logsumexp_k(alpha_{t-1}[k, b] + trans[j, k]),
alpha_0        = x_0 + orig.

Device recursion runs in the exp domain: with ea_t = exp(alpha_t - D_t[b]),
the step is one matmul + one elementwise multiply:

    S_t  = ETT @ ea_{t-1}              # ETT[k, j] = exp(trans[j, k])
    ea_t = exp(x_t) * S_t * r_t        # r_t = RSCALE * rhat every REN steps
    D_t  = D_{t-1} - ln r_t            # rhat recorded in bf16; recorded ==
                                       # applied exactly, so the approximate
                                       # reciprocal costs nothing in accuracy

Per-core layout stacks two 32-column batch chains on the 128 partitions
(chain a=0 on partitions 0:64, a=1 on 64:128) with a block-diagonal ETT,
so each timestep is a single [128x128]x[128,32] matmul and a single
[128,32] DVE multiply -- the serial chain is latency-bound, so fewer,
uniform instructions per step win.  Renorm prep (sigma matmul from the
te-3 state, fast reciprocal, bf16 record, broadcast matmul, w-scale) is
pipelined over the 3 steps before each event so it never blocks the chain.

The tag-score side is computed from host-built integer histograms
(count[next, cur] of transition pairs, plus first-tag counts): the device
does sum(count * ptab) where ptab = [trans^T | orig]; parameters are only
ever touched on device.

Sharding: data-parallel over batch, 64 rows per core; per-core partial
sums combined on the host (pure index constants only).
"""

import sys

sys.path.insert(0, "/opt/trn_rl_repo")

import numpy as np
import ml_dtypes

import concourse.bass as bass
import concourse.tile as tile
from concourse import bacc, mybir
from concourse.bass_utils import run_bass_kernel_spmd

# Problem constants (hardcoded per the task contract).
B, T, C = 512, 512, 64
M = 8            # cores
BL = B // M      # 64 batch rows per core
GW = 32          # columns per chain (2 chains stacked on partitions)
REN = 8          # renormalize every REN steps
NEVT = T // REN - 1          # 63 renorm events (t = 8, 16, ..., 504)
RQ = NEVT        # one [2, GW] recip-history block per event, partition base 0
RSCALE = 2.0 ** -40          # renorm down-scale, exact in bf16
LN_RSCALE = float(-40.0 * np.log(2.0))
CHUNK = 32       # timesteps of x per DMA chunk
SUB = 8          # timesteps per exp sub-activation
PAD_VAL = -1
NS = 4           # interleaved batch-column substreams per timestep
SW = GW // NS    # 8 columns per substream

f32 = mybir.dt.float32
bf16 = mybir.dt.bfloat16
u16 = mybir.dt.uint16
i16 = mybir.dt.int16
AF = mybir.ActivationFunctionType
ALU = mybir.AluOpType

# packed f32 const columns: trT | orig | ptab | cnt | parm
PK_TR, PK_OR, PK_PT, PK_CN, PK_PA = 0, 64, 65, 130, 195
PKF_W = 195 + 2 * BL
# packed bf16 const columns: b2 | ones2
PKB_W = 130

_CACHE = {}


def build_program(bench_reps=1):
    key = ("nc", bench_reps)
    if key in _CACHE:
        return _CACHE[key]
    nc = bacc.Bacc("TRN2", target_bir_lowering=False, debug=False)

    xT = nc.declare_dram_parameter("xT", [128, T * GW], f32, isOutput=False)
    pkf = nc.declare_dram_parameter("pkf", [128, PKF_W], f32, isOutput=False)
    pkb = nc.declare_dram_parameter("pkb", [128, PKB_W], bf16, isOutput=False)
    sidx = nc.declare_dram_parameter("sidx", [128, 4], i16, isOutput=False)
    cutm = nc.declare_dram_parameter("cutm", [128, RQ * GW], f32, isOutput=False)
    res = nc.declare_dram_parameter("res", [1, 8], f32, isOutput=True)

    NCH = T // CHUNK

    with tile.TileContext(nc) as tc:
        with (
            tc.tile_pool(name="const", bufs=1) as const,
            tc.tile_pool(name="hist", bufs=1) as histp,
            tc.tile_pool(name="xc", bufs=3) as xcpool,
            tc.tile_pool(name="xe", bufs=3) as xepool,
            tc.tile_pool(name="w", bufs=2) as wpool,
            tc.tile_pool(name="post", bufs=1) as post,
            tc.tile_pool(name="psS", bufs=4, space="PSUM") as psS,
            tc.tile_pool(name="psAux", bufs=1, space="PSUM") as psAux,
            tc.tile_pool(name="psFin", bufs=1, space="PSUM") as psFin,
        ):
            # ---- packed constants (sidx/cutm land later) ----
            # trT+orig split out front so ett/eo (and hence step 0) never
            # wait on the bulky histogram/mask columns.
            pkf_s = const.tile([128, PKF_W], f32, tag="pkf")
            nc.sync.dma_start(pkf_s[:, 0 : PK_PT], pkf[:, 0 : PK_PT])
            nc.sync.dma_start(pkf_s[:, PK_PT:], pkf[:, PK_PT:])
            pkb_s = const.tile([128, PKB_W], bf16, tag="pkb")
            nc.sync.dma_start(pkb_s[:], pkb[:])

            trT_v = pkf_s[0:C, PK_TR : PK_TR + C]
            orig_v = pkf_s[:, PK_OR : PK_OR + 1]
            ptab_v = pkf_s[0:C, PK_PT : PK_PT + C + 1]
            cnt_v = pkf_s[0:C, PK_CN : PK_CN + C + 1]
            parm_v = pkf_s[:, PK_PA : PK_PA + 2 * BL]
            b2_v = pkb_s[:, 0:128]
            ones2_v = pkb_s[:, 128:130]

            eo = const.tile([128, 1], f32, tag="eo")
            nc.scalar.activation(eo[:], orig_v, AF.Exp)

            # Block-diagonal exp(trans)^T: chain a on partitions a*64..a*64+63.
            ett = const.tile([128, 128], bf16, tag="ett")
            nc.vector.memset(ett[:], 0.0)
            nc.scalar.activation(ett[0:C, 0:C], trT_v, AF.Exp)
            nc.scalar.activation(ett[C:128, C:128], trT_v, AF.Exp)

            ones_col = const.tile([128, 1], f32, tag="ones_col")
            nc.vector.memset(ones_col[:], 1.0)

            # recip history: event r at partitions 0:2, columns [r*GW,(r+1)*GW).
            # Every slot is written exactly once, so no preset needed.
            rhist = const.tile([2, RQ * GW], bf16, tag="rhist")

            itc = const.tile([1, 1], f32, tag="itc")
            nc.vector.memset(itc[:], 0.0)

            # ea history: step t at columns [t*GW, (t+1)*GW).
            hist = histp.tile([128, T * GW], bf16, tag="hist")

            # ---- tag-score from host histograms (independent of recursion) ----
            gmul = post.tile([C, C + 1], f32, tag="gmul")
            gacc = post.tile([C, 1], f32, tag="gacc")
            nc.vector.scalar_tensor_tensor(
                gmul[:], cnt_v, 1.0, ptab_v,
                op0=ALU.mult, op1=ALU.mult, accum_out=gacc[:],
            )
            # single PSUM bank for the three tiny final matmul outputs
            fin = psFin.tile([1, 8], f32, tag="fin")
            btot = fin[0:1, 0:1]
            nc.tensor.matmul(
                btot, ones_col[0:C, :], gacc[:], start=True, stop=True
            )
            # single PSUM bank shared by the renorm sigma + broadcast matmuls
            aux = psAux.tile([128, 2 * GW], f32, tag="aux")

            def emit_recursion():
                chunk_tiles = {}

                def emit_chunk_dma(ci, split=1):
                    xc = xcpool.tile([128, CHUNK * GW], f32, tag="xc")
                    w = CHUNK * GW // split
                    for s in range(split):
                        nc.sync.dma_start(
                            xc[:, s * w : (s + 1) * w],
                            xT[:, ci * CHUNK * GW + s * w : ci * CHUNK * GW + (s + 1) * w],
                        )
                    xe = xepool.tile([128, CHUNK * GW], bf16, tag="xe")
                    chunk_tiles[ci] = (xc, xe)

                def emit_subexp(blk):
                    # blk covers steps [blk*SUB, (blk+1)*SUB)
                    if blk * SUB >= T:
                        return
                    ci, sub = divmod(blk, CHUNK // SUB)
                    xc, xe = chunk_tiles[ci]
                    lo, hi = sub * SUB * GW, (sub + 1) * SUB * GW
                    nc.scalar.activation(xe[:, lo:hi], xc[:, lo:hi], AF.Exp)

                # chunk 0 lands in 4 sub-DMAs so exp/step-0 start early
                emit_chunk_dma(0, split=2)
                emit_chunk_dma(1)
                for blk in range(3):
                    emit_subexp(blk)

                # ---- t = 0: ea_0 = exp(x_0) * exp(orig) ----
                xe0 = chunk_tiles[0][1]
                nc.vector.tensor_scalar_mul(hist[:, 0:GW], xe0[:, 0:GW], eo[:])

                wt = None
                rr = None
                sig = None
                for t in range(1, T):
                    ci, off = divmod(t, CHUNK)
                    if off == 0 and ci + 1 < NCH:
                        emit_chunk_dma(ci + 1)
                    if t % SUB == 0:
                        emit_subexp(t // SUB + 2)

                    xecur = chunk_tiles[ci][1]
                    xoff = off * GW
                    if t % REN == 0 and t <= REN * NEVT:
                        win = wt[:]
                    else:
                        win = xecur[:, xoff : xoff + GW]

                    ph = t % REN
                    te = t + (REN - ph)
                    prep = te <= REN * NEVT

                    S = psS.tile([128, GW], f32, tag="S")
                    nc.tensor.matmul(
                        S[:], ett[:], hist[:, (t - 1) * GW : t * GW],
                        start=True, stop=True,
                    )
                    if prep and ph == REN - 1:
                        # w-scale for the event at te=t+1, emitted BEFORE this
                        # step's TT: its inputs (Rb from te-2, xe) are ready,
                        # so DVE runs it inside the matmul-in-flight window
                        # and the event step pays no extra DVE latency.
                        nci, noff = divmod(t + 1, CHUNK)
                        wt = wpool.tile([128, GW], bf16, tag="wt")
                        nc.vector.tensor_mul(
                            wt[:],
                            chunk_tiles[nci][1][:, noff * GW : (noff + 1) * GW],
                            aux[:, GW : 2 * GW],
                        )
                    nc.vector.tensor_mul(hist[:, t * GW : (t + 1) * GW], S[:], win)

                    # renorm prep for the event at te = next multiple of REN,
                    # pipelined over the 4 preceding steps (sigma from te-4).
                    if prep:
                        r = te // REN - 1
                        rcol = r * GW
                        if ph == REN - 4:
                            sig = aux[0:2, 0:GW]
                            nc.tensor.matmul(
                                sig, ones2_v, hist[:, t * GW : (t + 1) * GW],
                                start=True, stop=True,
                            )
                            rr = wpool.tile([2, GW], f32, tag="rr")
                        elif ph == REN - 3:
                            nc.vector.reciprocal_approx_fast(rr[:], aux[0:2, 0:GW])
                            nc.vector.tensor_copy(
                                rhist[0:2, rcol : rcol + GW], rr[:]
                            )
                        elif ph == REN - 2:
                            nc.tensor.matmul(
                                aux[:, GW : 2 * GW],
                                b2_v[0:2, :],
                                rhist[0:2, rcol : rcol + GW],
                                start=True, stop=True,
                            )

            def emit_rep():
                nc.vector.tensor_scalar_add(itc[:], itc[:], 1.0)
                emit_recursion()

            if bench_reps == 1:
                emit_rep()
            else:
                with tc.For_i(0, bench_reps, 1):
                    emit_rep()

            # ---- tail: recip-history reduction + final extraction ----
            cutm_s = const.tile([128, RQ * GW], f32, tag="cutm")
            nc.sync.dma_start(cutm_s[:], cutm[:])
            sidx_r = const.tile([128, 4], i16, tag="sidx_r")
            nc.sync.dma_start(sidx_r[:], sidx[:])
            # the gpsimd gather wants all inputs written by one engine: DVE stage
            sidx_s = const.tile([128, 4], i16, tag="sidx")
            nc.vector.tensor_copy(sidx_s[:], sidx_r[:])

            # 4 sub-passes: quarters become ready as their events complete,
            # so the scheduler hoists the early ones into recursion slack.
            lnr = post.tile([2, RQ * GW], f32, tag="lnr")
            masked = post.tile([2, RQ * GW], f32, tag="masked")
            racc = post.tile([2, 4], f32, tag="racc")
            NQ4 = (RQ * GW) // 4
            for q in range(4):
                lo, hi = q * NQ4, (q + 1) * NQ4
                nc.scalar.activation(lnr[0:2, lo:hi], rhist[0:2, lo:hi], AF.Ln)
                nc.vector.scalar_tensor_tensor(
                    masked[0:2, lo:hi], lnr[0:2, lo:hi], 1.0, cutm_s[0:2, lo:hi],
                    op0=ALU.mult, op1=ALU.mult,
                    accum_out=racc[0:2, q : q + 1],
                )

            # ap_gather of column PAIRS (bf16 needs d*size % 4 == 0):
            # out pair-slot b holds hist cols 2*pi, 2*pi+1 for pi = idx//2;
            # the wanted column's parity is folded into the parm mask.
            snap = post.tile([128, 2 * BL], bf16, tag="snap")
            nc.gpsimd.ap_gather(
                snap[:].rearrange("p (n d) -> p n d", d=2),
                hist[:].rearrange("p (n d) -> p n d", d=2),
                sidx_s[:],
                channels=128, num_elems=T * GW // 2, d=2, num_idxs=BL,
            )
            snapln = post.tile([128, 2 * BL], f32, tag="snapln")
            nc.scalar.activation(snapln[:], snap[:], AF.Ln)
            snapsel = post.tile([128, 2 * BL], f32, tag="snapsel")
            sacc = post.tile([128, 1], f32, tag="sacc")
            nc.vector.scalar_tensor_tensor(
                snapsel[:], snapln[:], 1.0, parm_v,
                op0=ALU.mult, op1=ALU.mult, accum_out=sacc[:],
            )

            nA = fin[0:1, 1:2]
            nc.tensor.matmul(nA, ones_col[:], sacc[:], start=True, stop=True)
            nB = fin[0:1, 2:6]
            nc.tensor.matmul(nB, ones_col[0:2, :], racc[:], start=True, stop=True)

            out_s = post.tile([1, 8], f32, tag="out")
            nc.vector.tensor_copy(out_s[0:1, 0:1], btot)
            nc.vector.tensor_copy(out_s[0:1, 1:2], nA)
            nc.vector.tensor_copy(out_s[0:1, 2:6], nB)
            nc.vector.tensor_copy(out_s[0:1, 6:7], itc[:])
            nc.vector.tensor_copy(out_s[0:1, 7:8], itc[:])
            nc.sync.dma_start(res[:], out_s[:])

    nc.compile()
    _CACHE[key] = nc
    return nc


def host_inputs(pad_x, transition_scores, origination_scores, pad_y, batch_sizes):
    """Shard + lay out the full inputs into 8 per-core input maps.

    Host work is limited to data movement and integer index preprocessing;
    every floating-point op on learned parameters / activations runs on
    device.  Returns (in_maps, nev_consts)."""
    pad_x = np.ascontiguousarray(np.asarray(pad_x, dtype=np.float32))
    trans = np.ascontiguousarray(np.asarray(transition_scores, dtype=np.float32))
    origv = np.ascontiguousarray(np.asarray(origination_scores, dtype=np.float32))
    pad_y = np.asarray(pad_y)
    batch_sizes = np.asarray(batch_sizes)

    # x: xT[c][a*64 + k, t*32 + cc] = pad_x[c*64 + a*32 + cc, t, k]
    xr = pad_x.reshape(M, 2, GW, T, C).transpose(0, 1, 4, 3, 2)
    xT = np.ascontiguousarray(xr).reshape(M, 128, T * GW)

    y = np.where(pad_y == PAD_VAL, 0, pad_y).astype(np.int64)
    tau = batch_sizes.astype(np.int64) - 1

    pkb = np.zeros((128, PKB_W), np.float32)
    pkb[0, 0:64] = RSCALE          # b2 row 0 -> chain A
    pkb[1, 64:128] = RSCALE        # b2 row 1 -> chain B
    pkb[0:64, 128] = 1.0           # ones2 col 0
    pkb[64:128, 129] = 1.0         # ones2 col 1
    pkb = np.ascontiguousarray(pkb.astype(ml_dtypes.bfloat16))

    # parm[chain_half(b), 2*b + parity(b)] = 1; gathered pairs elsewhere 0
    parm = np.zeros((128, 2 * BL), np.float32)
    for b in range(BL):
        a = b // GW
        parm[a * 64 : (a + 1) * 64, 2 * b + (b % 2)] = 1.0

    in_maps = []
    nevs = []
    for c in range(M):
        yc = y[c * BL : (c + 1) * BL]
        # count[next, cur] histogram + first-tag histogram (integer only)
        pair = (yc[:, 1:] * C + yc[:, :-1]).reshape(-1)
        cntm = np.bincount(pair, minlength=C * C).astype(np.float32).reshape(C, C)
        ho = np.bincount(yc[:, 0], minlength=C).astype(np.float32).reshape(C, 1)

        pkf = np.zeros((128, PKF_W), np.float32)
        pkf[0:C, PK_TR : PK_TR + C] = trans.T
        pkf[:, PK_OR] = np.concatenate([origv, origv])
        pkf[0:C, PK_PT : PK_PT + C] = trans.T
        pkf[0:C, PK_PT + C] = origv
        pkf[0:C, PK_CN : PK_CN + C] = cntm
        pkf[0:C, PK_CN + C] = ho[:, 0]
        pkf[:, PK_PA : PK_PA + 2 * BL] = parm

        tauc = tau[c * BL : (c + 1) * BL]
        # pair index of column tau*GW + (b%GW); parity = b%2 (GW is even)
        idxp = ((tauc * GW + (np.arange(BL) % GW)) // 2).astype(np.int16)
        blk = idxp.reshape(4, 16).T  # wrapped per 16 partitions
        sidx = np.ascontiguousarray(np.tile(blk, (8, 1)))

        # cutm[a, r*GW+cc] = 1 iff event r has REN*(r+1) <= tau of batch col
        # b = a*GW + cc
        cutm = np.zeros((128, RQ * GW), np.float32)
        for r in range(NEVT):
            t_r = REN * (r + 1)
            for a in range(2):
                bvals = tauc[a * GW : (a + 1) * GW]
                cutm[a, r * GW : (r + 1) * GW] = (t_r <= bvals).astype(np.float32)

        nevs.append(float(np.minimum(tauc // REN, NEVT).sum()))

        in_maps.append(
            {
                "xT": np.ascontiguousarray(xT[c]),
                "pkf": np.ascontiguousarray(pkf),
                "pkb": pkb,
                "sidx": sidx,
                "cutm": np.ascontiguousarray(cutm),
            }
        )
    return in_maps, nevs


def combine(results, nevs):
    total = 0.0
    for r, nev in zip(results, nevs):
        v = np.asarray(r["res"], dtype=np.float64).reshape(-1)
        # loss_core = score - sum_b normalizer_b
        #           = v0 - (v1 - C*(sum(v2..v5) + ln(RSCALE)*nev))
        total += v[0] - v[1] + C * (v[2] + v[3] + v[4] + v[5] + LN_RSCALE * nev)
    return np.asarray(total, dtype=np.float32)


def kernel(pad_x, transition_scores, origination_scores, pad_y, batch_sizes):
    nc = build_program()
    in_maps, nevs = host_inputs(
        pad_x, transition_scores, origination_scores, pad_y, batch_sizes
    )
    out = run_bass_kernel_spmd(nc, in_maps, core_ids=list(range(M)))
    return combine(out.results, nevs)

